# revision 7
# baseline (speedup 1.0000x reference)
"""Trainium2 Bass kernel for a 3-layer dense transformer LM (B=8, T=1024,
D=384, H=6, V=32000) returning (logits, loss).

Sharding: pure data-parallel over batch — core c computes batch element c
end-to-end (no collectives). Host assembles logits and averages the
per-token NLLs.

All matmuls run in bf16 with f32 PSUM accumulation. Attention is computed
in transposed-score orientation (scoresT[u,t]) so softmax needs no PE
transposes: the denominator comes from an appended ones-column on V, and
normalization is applied to attT (folded past the linear AV matmul).
"""

import os
import sys

for _p in ("/root/.axon_site/_ro/trn_rl_repo", "/opt/trn_rl_repo"):
    if os.path.isdir(_p) and _p not in sys.path:
        sys.path.append(_p)

import ml_dtypes
import numpy as np

import concourse.bass as bass
import concourse.tile as tile
from concourse import bacc, mybir
from concourse.bass import IndirectOffsetOnAxis
from concourse.bass_utils import run_bass_kernel_spmd
from concourse.masks import make_identity

F32 = mybir.dt.float32
BF16 = mybir.dt.bfloat16
I32 = mybir.dt.int32
AF = mybir.ActivationFunctionType
ALU = mybir.AluOpType

P = 128
D = 384
KO = D // P  # 3
H = 6
HS = 64
T = 1024
NT = T // P  # 8 token tiles
NL = 3
V = 32000
FF = 4 * D  # 1536
FM = FF // P  # 12
G = T // 512  # 2 halves of the token axis
VTW = 512
NVT = (V + VTW - 1) // VTW  # 63 (62 full + 1 x 256)
NCORES = 8
EPS = 1e-5
SCALE = HS**-0.5  # 1/8
NEG_BIG = -1e30


def build_program(has_blm=True, has_b2=True):
    nc = bacc.Bacc(
        "TRN2", target_bir_lowering=False, debug=False, enable_asserts=False
    )

    # ---- DRAM I/O -------------------------------------------------------
    idx32 = nc.dram_tensor("idx32", [T, 1], I32, kind="ExternalInput").ap()
    tgt32 = nc.dram_tensor("tgt32", [T, 1], I32, kind="ExternalInput").ap()
    tok = nc.dram_tensor("tok", [V, D], F32, kind="ExternalInput").ap()
    pos = nc.dram_tensor("pos", [T, D], F32, kind="ExternalInput").ap()

    lw_dram = []
    for l in range(NL):
        lw_dram.append(
            {
                "wq": nc.dram_tensor(f"wq{l}", [KO, P, D], BF16, kind="ExternalInput").ap(),
                "wk": nc.dram_tensor(f"wk{l}", [KO, P, D], BF16, kind="ExternalInput").ap(),
                "wv": nc.dram_tensor(f"wv{l}", [KO, P, D], BF16, kind="ExternalInput").ap(),
                "wo": nc.dram_tensor(f"wo{l}", [H, HS, D], BF16, kind="ExternalInput").ap(),
                "w1": nc.dram_tensor(f"w1_{l}", [KO, P, FF], BF16, kind="ExternalInput").ap(),
                "w2": nc.dram_tensor(f"w2_{l}", [FM, P, D], BF16, kind="ExternalInput").ap(),
                "b1": nc.dram_tensor(f"b1_{l}", [FM, P], F32, kind="ExternalInput").ap(),
                "b2row": nc.dram_tensor(f"b2row{l}", [1, D], BF16, kind="ExternalInput").ap(),
                "g1": nc.dram_tensor(f"g1_{l}", [D], F32, kind="ExternalInput").ap(),
                "be1": nc.dram_tensor(f"be1_{l}", [D], F32, kind="ExternalInput").ap(),
                "g2": nc.dram_tensor(f"g2_{l}", [D], F32, kind="ExternalInput").ap(),
                "be2": nc.dram_tensor(f"be2_{l}", [D], F32, kind="ExternalInput").ap(),
            }
        )
    gf = nc.dram_tensor("gf", [D], F32, kind="ExternalInput").ap()
    bff = nc.dram_tensor("bff", [D], F32, kind="ExternalInput").ap()
    wlm = nc.dram_tensor("wlm", [KO, P, V], BF16, kind="ExternalInput").ap()
    wlmT = nc.dram_tensor("wlmT", [V, D], F32, kind="ExternalInput").ap()
    blmcol = nc.dram_tensor("blmcol", [V, 1], F32, kind="ExternalInput").ap()
    blmrow = nc.dram_tensor("blmrow", [1, V], BF16, kind="ExternalInput").ap()

    logits = nc.dram_tensor("logits", [T, V], F32, kind="ExternalOutput").ap()
    nll = nc.dram_tensor("nll", [T, 1], F32, kind="ExternalOutput").ap()

    with tile.TileContext(nc) as tc:
        _build_body(
            nc, tc, idx32, tgt32, tok, pos, lw_dram, gf, bff, wlm, wlmT,
            blmcol, blmrow, logits, nll, has_blm, has_b2,
        )
    nc.compile()
    return nc


def _build_body(
    nc, tc, idx32, tgt32, tok, pos, lw_dram, gf, bff, wlm, wlmT, blmcol,
    blmrow, logits, nll, has_blm, has_b2,
):
    from contextlib import ExitStack

    ctx = ExitStack()
    with ctx:
        const = ctx.enter_context(tc.tile_pool(name="const", bufs=1))
        lw = ctx.enter_context(tc.tile_pool(name="lw", bufs=1))
        act = ctx.enter_context(tc.tile_pool(name="act", bufs=2))
        big = ctx.enter_context(tc.tile_pool(name="big", bufs=1))
        bigp = ctx.enter_context(tc.tile_pool(name="bigp", bufs=2))
        small = ctx.enter_context(tc.tile_pool(name="small", bufs=4))
        wstream = ctx.enter_context(tc.tile_pool(name="wstream", bufs=3))
        lsb = ctx.enter_context(tc.tile_pool(name="lsb", bufs=4))
        esc = ctx.enter_context(tc.tile_pool(name="esc", bufs=3))
        ps_big = ctx.enter_context(tc.tile_pool(name="ps_big", bufs=3, space="PSUM"))
        ps_att = ctx.enter_context(tc.tile_pool(name="ps_att", bufs=2, space="PSUM"))
        ps_y = ctx.enter_context(tc.tile_pool(name="ps_y", bufs=2, space="PSUM"))
        ps_tr = ctx.enter_context(tc.tile_pool(name="ps_tr", bufs=1, space="PSUM"))

        # ---- constants ---------------------------------------------------
        ident_bf = const.tile([P, P], BF16, tag="ident")
        make_identity(nc, ident_bf[:])
        ones_row = const.tile([1, P], BF16, tag="ones_row")
        nc.vector.memset(ones_row[:], 1.0)
        eps_t = const.tile([P, 1], F32, tag="eps")
        nc.vector.memset(eps_t[:], EPS)
        gfb = const.tile([P, D], F32, tag="gfb")
        nc.sync.dma_start(gfb[:], gf.partition_broadcast(P))
        bfb = const.tile([P, D], F32, tag="bfb")
        nc.sync.dma_start(bfb[:], bff.partition_broadcast(P))

        x_tiles = [const.tile([P, D], F32, tag=f"x{i}", name=f"x{i}") for i in range(NT)]
        s_tiles = [const.tile([P, NVT], F32, tag=f"s{i}", name=f"s{i}") for i in range(NT)]
        tgt_tiles = [const.tile([P, 1], F32, tag=f"tgt{i}", name=f"tgt{i}") for i in range(NT)]

        # ---- helpers -----------------------------------------------------
        def layernorm(src_ap, g_b, be_b, out_ap):
            """out = (src - mean)/sqrt(var+eps) * g + be, written to out_ap."""
            stats = small.tile([P, 6], F32, tag="bnstats")
            nc.vector.bn_stats(stats[:], src_ap)
            mv = small.tile([P, 2], F32, tag="mv")
            nc.vector.bn_aggr(mv[:], stats[:])
            rstd = small.tile([P, 1], F32, tag="rstd")
            nc.scalar.activation(rstd[:], mv[:, 1:2], AF.Sqrt, bias=eps_t[:], scale=1.0)
            nc.vector.reciprocal(rstd[:], rstd[:])
            nmr = small.tile([P, 1], F32, tag="nmr")
            nc.vector.tensor_tensor(nmr[:], mv[:, 0:1], rstd[:], ALU.mult)
            nc.vector.tensor_scalar_mul(nmr[:], nmr[:], -1.0)
            htmp = act.tile([P, D], F32, tag="htmp")
            nc.scalar.activation(htmp[:], src_ap, AF.Identity, bias=nmr[:], scale=rstd[:])
            nc.vector.tensor_tensor(htmp[:], htmp[:], g_b[:], ALU.mult)
            nc.vector.tensor_tensor(out_ap, htmp[:], be_b[:], ALU.add)

        def transpose_to(dst_ap, src_ap):
            """dst[128,128] (bf16 sbuf) = src[128,128] (bf16 sbuf) transposed.

            Regular matmul against the identity (out = src.T @ I): the
            dedicated is_transpose path requires a bf16 PSUM tile, which
            crashes the device (NRT_EXEC_UNIT_UNRECOVERABLE)."""
            pt = ps_tr.tile([P, P], F32, tag="ptr", space="PSUM")
            nc.tensor.matmul(pt[:], lhsT=src_ap, rhs=ident_bf[:], start=True, stop=True)
            nc.vector.tensor_copy(dst_ap, pt[:])

        # ---- embedding ---------------------------------------------------
        es = nc.named_scope("embed"); es.__enter__()
        for i in range(NT):
            sl = slice(i * P, (i + 1) * P)
            idx_t = small.tile([P, 1], I32, tag="idx")
            nc.sync.dma_start(idx_t[:], idx32[sl, :])
            xg = act.tile([P, D], F32, tag="xg")
            nc.gpsimd.indirect_dma_start(
                out=xg[:],
                out_offset=None,
                in_=tok[:],
                in_offset=IndirectOffsetOnAxis(ap=idx_t[:, :1], axis=0),
            )
            pos_t = act.tile([P, D], F32, tag="pos_t")
            nc.sync.dma_start(pos_t[:], pos[sl, :])
            nc.vector.tensor_tensor(x_tiles[i][:], xg[:], pos_t[:], ALU.add)

        es.__exit__(None, None, None)
        # ---- transformer layers -----------------------------------------
        for l in range(NL):
            ls = nc.named_scope(f"layer{l}"); ls.__enter__()
            d = lw_dram[l]
            wq_sb = lw.tile([P, KO, D], BF16, tag="wq")
            nc.sync.dma_start(wq_sb[:], d["wq"].rearrange("ko p n -> p ko n"))
            wk_sb = lw.tile([P, KO, D], BF16, tag="wk")
            nc.sync.dma_start(wk_sb[:], d["wk"].rearrange("ko p n -> p ko n"))
            wv_sb = lw.tile([P, KO, D], BF16, tag="wv")
            nc.sync.dma_start(wv_sb[:], d["wv"].rearrange("ko p n -> p ko n"))
            wo_sb = lw.tile([HS, H, D], BF16, tag="wo")
            nc.sync.dma_start(wo_sb[:], d["wo"].rearrange("h s n -> s h n"))
            w1_sb = lw.tile([P, KO, FF], BF16, tag="w1")
            nc.sync.dma_start(w1_sb[:], d["w1"].rearrange("ko p n -> p ko n"))
            w2_sb = lw.tile([P, FM, D], BF16, tag="w2")
            nc.sync.dma_start(w2_sb[:], d["w2"].rearrange("m p n -> p m n"))
            b1_sb = lw.tile([P, FM], F32, tag="b1")
            nc.sync.dma_start(b1_sb[:], d["b1"].rearrange("m p -> p m"))
            b2row_sb = lw.tile([1, D], BF16, tag="b2row")
            nc.sync.dma_start(b2row_sb[:], d["b2row"][:])
            g1b = lw.tile([P, D], F32, tag="g1b")
            nc.sync.dma_start(g1b[:], d["g1"].partition_broadcast(P))
            be1b = lw.tile([P, D], F32, tag="be1b")
            nc.sync.dma_start(be1b[:], d["be1"].partition_broadcast(P))
            g2b = lw.tile([P, D], F32, tag="g2b")
            nc.sync.dma_start(g2b[:], d["g2"].partition_broadcast(P))
            be2b = lw.tile([P, D], F32, tag="be2b")
            nc.sync.dma_start(be2b[:], d["be2"].partition_broadcast(P))

            # LN1 + transpose h into hT [P, KO, T]
            hT = big.tile([P, KO, T], BF16, tag="hT")
            for i in range(NT):
                h_bf = act.tile([P, D], BF16, tag="h_bf")
                layernorm(x_tiles[i][:], g1b, be1b, h_bf[:])
                for ko in range(KO):
                    transpose_to(
                        hT[:, ko, i * P : (i + 1) * P],
                        h_bf[:, ko * P : (ko + 1) * P],
                    )

            # qT, kT: [P, KO, T] bf16; partition = col within head-pair m
            # (head 2m on partitions 0:64, head 2m+1 on 64:128)
            qT = big.tile([P, KO, T], BF16, tag="qT")
            kT = big.tile([P, KO, T], BF16, tag="kT")
            for w_sb, oT in ((wq_sb, qT), (wk_sb, kT)):
                for m in range(KO):  # head pair m -> heads 2m, 2m+1
                    for g in range(G):
                        psq = ps_big.tile([P, 512], F32, tag="ps512", space="PSUM")
                        for ko in range(KO):
                            nc.tensor.matmul(
                                psq[:],
                                lhsT=w_sb[:, ko, m * P : (m + 1) * P],
                                rhs=hT[:, ko, g * 512 : (g + 1) * 512],
                                start=(ko == 0),
                                stop=(ko == KO - 1),
                            )
                        gs = slice(g * 512, (g + 1) * 512)
                        nc.scalar.copy(oT[:, m, gs], psq[:])

            # v (token-major) with appended ones column: [P, NT, H, HS+1]
            vaug = big.tile([P, NT, H, HS + 1], BF16, tag="vaug")
            nc.vector.memset(vaug[:, :, :, HS : HS + 1], 1.0)
            for i in range(NT):
                psv_full = ps_big.tile([P, 512], F32, tag="ps512", space="PSUM")
                psv = psv_full[:, :D]
                for ko in range(KO):
                    nc.tensor.matmul(
                        psv[:],
                        lhsT=hT[:, ko, i * P : (i + 1) * P],
                        rhs=wv_sb[:, ko, :],
                        start=(ko == 0),
                        stop=(ko == KO - 1),
                    )
                nc.vector.tensor_copy(
                    vaug[:, i, :, 0:HS],
                    psv[:].rearrange("p (h s) -> p h s", h=H),
                )

            # attention, head by head
            attT = big.tile([HS, H, T], BF16, tag="attT")
            for h in range(H):
                probsT = bigp.tile(
                    [P, NT, T], BF16, tag="probsT", name=f"probsT{h}"
                )
                for g in range(G):
                    jmax = 4 * g + 3
                    gs = slice(g * 512, (g + 1) * 512)
                    for j in range(jmax + 1):
                        tlo = max(g * 512, j * P)
                        n = (g + 1) * 512 - tlo
                        pss = ps_big.tile([P, 512], F32, tag="ps512", space="PSUM")
                        po = (h % 2) * HS
                        m_h = h // 2
                        nc.tensor.matmul(
                            pss[:, :n],
                            lhsT=kT[po : po + HS, m_h, j * P : (j + 1) * P],
                            rhs=qT[po : po + HS, m_h, tlo : (g + 1) * 512],
                            start=True,
                            stop=True,
                        )
                        if j * P >= g * 512:  # diagonal block present
                            if j * P > g * 512:
                                nc.vector.memset(probsT[:, j, g * 512 : j * P], 0.0)
                            sdiag = small.tile([P, P], F32, tag="sdiag")
                            nc.scalar.copy(sdiag[:], pss[:, 0:P])
                            nc.gpsimd.affine_select(
                                out=sdiag[:],
                                in_=sdiag[:],
                                pattern=[[1, P]],
                                compare_op=ALU.is_ge,
                                fill=NEG_BIG,
                                base=0,
                                channel_multiplier=-1,
                            )
                            nc.scalar.activation(
                                probsT[:, j, tlo : tlo + P], sdiag[:], AF.Exp,
                                scale=SCALE,
                            )
                            if n > P:
                                nc.scalar.activation(
                                    probsT[:, j, tlo + P : (g + 1) * 512],
                                    pss[:, P:n], AF.Exp, scale=SCALE,
                                )
                        else:
                            nc.scalar.activation(
                                probsT[:, j, gs], pss[:, :n], AF.Exp, scale=SCALE
                            )
                    # AV: attT_aug[s,t] (+ rowsum in row HS) over u blocks
                    psa = ps_att.tile([HS + 1, 512], F32, tag="psa", space="PSUM")
                    for j in range(jmax + 1):
                        nc.tensor.matmul(
                            psa[:],
                            lhsT=vaug[:, j, h, :],
                            rhs=probsT[:, j, gs],
                            start=(j == 0),
                            stop=(j == jmax),
                        )
                    rs_row = small.tile([1, 512], F32, tag="rs_row")
                    nc.scalar.copy(rs_row[:], psa[HS : HS + 1, :])
                    rsB = small.tile([HS, 512], F32, tag="rsB")
                    nc.gpsimd.partition_broadcast(rsB[:], rs_row[:], channels=HS)
                    nc.vector.reciprocal(rsB[:], rsB[:])
                    nc.vector.tensor_tensor(
                        attT[:, h, gs], psa[0:HS, :], rsB[:], ALU.mult
                    )

            # output projection + residual
            for i in range(NT):
                psy = ps_y.tile([P, D], F32, tag="psy", space="PSUM")
                for h in range(H):
                    nc.tensor.matmul(
                        psy[:],
                        lhsT=attT[:, h, i * P : (i + 1) * P],
                        rhs=wo_sb[:, h, :],
                        start=(h == 0),
                        stop=(h == H - 1),
                    )
                nc.vector.tensor_tensor(x_tiles[i][:], x_tiles[i][:], psy[:], ALU.add)

            # FFW
            h2T = big.tile([P, KO, T], BF16, tag="hT")  # reuse hT slot
            for i in range(NT):
                h2_bf = act.tile([P, D], BF16, tag="h_bf")
                layernorm(x_tiles[i][:], g2b, be2b, h2_bf[:])
                for ko in range(KO):
                    transpose_to(
                        h2T[:, ko, i * P : (i + 1) * P],
                        h2_bf[:, ko * P : (ko + 1) * P],
                    )
            ff1T = big.tile([P, FM, T], BF16, tag="ff1T")
            for m in range(FM):
                for g in range(G):
                    psf = ps_big.tile([P, 512], F32, tag="ps512", space="PSUM")
                    for ko in range(KO):
                        nc.tensor.matmul(
                            psf[:],
                            lhsT=w1_sb[:, ko, m * P : (m + 1) * P],
                            rhs=h2T[:, ko, g * 512 : (g + 1) * 512],
                            start=(ko == 0),
                            stop=(ko == KO - 1),
                        )
                    nc.scalar.activation(
                        ff1T[:, m, g * 512 : (g + 1) * 512], psf[:], AF.Relu,
                        bias=b1_sb[:, m : m + 1], scale=1.0,
                    )
            for i in range(NT):
                psy2 = ps_y.tile([P, D], F32, tag="psy", space="PSUM")
                for m in range(FM):
                    nc.tensor.matmul(
                        psy2[:],
                        lhsT=ff1T[:, m, i * P : (i + 1) * P],
                        rhs=w2_sb[:, m, :],
                        start=(m == 0),
                        stop=(not has_b2 and m == FM - 1),
                    )
                if has_b2:
                    nc.tensor.matmul(
                        psy2[:], lhsT=ones_row[:], rhs=b2row_sb[:], start=False,
                        stop=True,
                    )
                nc.vector.tensor_tensor(x_tiles[i][:], x_tiles[i][:], psy2[:], ALU.add)

            ls.__exit__(None, None, None)
        # ---- final LN, target-logit dot ---------------------------------
        fs = nc.named_scope("finalln"); fs.__enter__()
        hfT = big.tile([P, KO, T], BF16, tag="hT")
        for i in range(NT):
            sl = slice(i * P, (i + 1) * P)
            hf = act.tile([P, D], F32, tag="hf")
            layernorm(x_tiles[i][:], gfb, bfb, hf[:])
            hf_bf = act.tile([P, D], BF16, tag="h_bf")
            nc.vector.tensor_copy(hf_bf[:], hf[:])
            for ko in range(KO):
                transpose_to(
                    hfT[:, ko, i * P : (i + 1) * P], hf_bf[:, ko * P : (ko + 1) * P]
                )
            tgi = small.tile([P, 1], I32, tag="idx")
            nc.sync.dma_start(tgi[:], tgt32[sl, :])
            wg = act.tile([P, D], F32, tag="wg")
            nc.gpsimd.indirect_dma_start(
                out=wg[:],
                out_offset=None,
                in_=wlmT[:],
                in_offset=IndirectOffsetOnAxis(ap=tgi[:, :1], axis=0),
            )
            bg = small.tile([P, 1], F32, tag="bg")
            nc.gpsimd.indirect_dma_start(
                out=bg[:],
                out_offset=None,
                in_=blmcol[:],
                in_offset=IndirectOffsetOnAxis(ap=tgi[:, :1], axis=0),
            )
            ttr_out = act.tile([P, D], F32, tag="ttr")
            tdot = small.tile([P, 1], F32, tag="tdot")
            nc.vector.tensor_tensor(ttr_out[:], hf[:], wg[:], ALU.mult)
            nc.vector.tensor_reduce(
                tdot[:], ttr_out[:], axis=mybir.AxisListType.X, op=ALU.add
            )
            nc.vector.tensor_tensor(tgt_tiles[i][:], tdot[:], bg[:], ALU.add)

        fs.__exit__(None, None, None)
        hs = nc.named_scope("lmhead"); hs.__enter__()
        for vt in range(NVT):
            c0 = vt * VTW
            w = min(VTW, V - c0)
            wlm_t = wstream.tile([P, KO, VTW], BF16, tag="wlm")
            nc.sync.dma_start(
                wlm_t[:, :, :w],
                wlm[:, :, c0 : c0 + w].rearrange("ko p n -> p ko n"),
            )
            if has_blm:
                blm_t = wstream.tile([1, VTW], BF16, tag="blmr")
                nc.sync.dma_start(blm_t[:, :w], blmrow[:, c0 : c0 + w])
            for i in range(NT):
                psl = ps_big.tile([P, VTW], F32, tag="ps512", space="PSUM")
                for ko in range(KO):
                    nc.tensor.matmul(
                        psl[:, :w],
                        lhsT=hfT[:, ko, i * P : (i + 1) * P],
                        rhs=wlm_t[:, ko, :w],
                        start=(ko == 0),
                        stop=(not has_blm and ko == KO - 1),
                    )
                if has_blm:
                    nc.tensor.matmul(
                        psl[:, :w],
                        lhsT=ones_row[:],
                        rhs=blm_t[:, :w],
                        start=False,
                        stop=True,
                    )
                lt = lsb.tile([P, VTW], F32, tag="lt")
                nc.vector.tensor_copy(lt[:, :w], psl[:, :w])
                nc.sync.dma_start(logits[i * P : (i + 1) * P, c0 : c0 + w], lt[:, :w])
                et = esc.tile([P, VTW], BF16, tag="et")
                nc.scalar.activation(
                    et[:, :w], psl[:, :w], AF.Exp,
                    accum_out=s_tiles[i][:, vt : vt + 1],
                )

        hs.__exit__(None, None, None)
        ts_ = nc.named_scope("loss"); ts_.__enter__()
        for i in range(NT):
            ssum = small.tile([P, 1], F32, tag="ssum")
            nc.vector.tensor_reduce(
                ssum[:], s_tiles[i][:], axis=mybir.AxisListType.X, op=ALU.add
            )
            lse = small.tile([P, 1], F32, tag="lse")
            nc.scalar.activation(lse[:], ssum[:], AF.Ln)
            nll_t = small.tile([P, 1], F32, tag="nll_t")
            nc.vector.tensor_tensor(nll_t[:], lse[:], tgt_tiles[i][:], ALU.subtract)
            nc.sync.dma_start(nll[i * P : (i + 1) * P, :], nll_t[:])
        ts_.__exit__(None, None, None)


# ---------------------------------------------------------------------------
# Host side
# ---------------------------------------------------------------------------

_PROGRAMS = {}


def _get_program(has_blm, has_b2):
    key = (has_blm, has_b2)
    if key not in _PROGRAMS:
        _PROGRAMS[key] = build_program(has_blm, has_b2)
    return _PROGRAMS[key]


def _prep_shared(params):
    bf = ml_dtypes.bfloat16
    f32 = np.float32
    out = {}
    out["tok"] = np.ascontiguousarray(np.asarray(params["tok"], f32))
    out["pos"] = np.ascontiguousarray(np.asarray(params["pos"], f32)[:T])
    for l, blk in enumerate(params["blocks"]):
        for nm in ("wq", "wk", "wv"):
            w = np.asarray(blk[nm], f32)  # [H, D, HS]
            w = np.transpose(w, (1, 0, 2)).reshape(D, H * HS)
            out[f"{nm}{l}"] = np.ascontiguousarray(w.reshape(KO, P, D).astype(bf))
        wo = np.asarray(blk["wo"], f32).reshape(H, HS, D)
        out[f"wo{l}"] = np.ascontiguousarray(wo.astype(bf))
        out[f"w1_{l}"] = np.ascontiguousarray(
            np.asarray(blk["w1"], f32).reshape(KO, P, FF).astype(bf)
        )
        out[f"w2_{l}"] = np.ascontiguousarray(
            np.asarray(blk["w2"], f32).reshape(FM, P, D).astype(bf)
        )
        out[f"b1_{l}"] = np.ascontiguousarray(
            np.asarray(blk["b1"], f32).reshape(FM, P)
        )
        out[f"b2row{l}"] = np.ascontiguousarray(
            np.asarray(blk["b2"], f32).reshape(1, D).astype(bf)
        )
        out[f"g1_{l}"] = np.ascontiguousarray(np.asarray(blk["g1"], f32))
        out[f"be1_{l}"] = np.ascontiguousarray(np.asarray(blk["be1"], f32))
        out[f"g2_{l}"] = np.ascontiguousarray(np.asarray(blk["g2"], f32))
        out[f"be2_{l}"] = np.ascontiguousarray(np.asarray(blk["be2"], f32))
    out["gf"] = np.ascontiguousarray(np.asarray(params["gf"], f32))
    out["bff"] = np.ascontiguousarray(np.asarray(params["bf"], f32))
    wlm = np.asarray(params["wlm"], f32)  # [D, V]
    out["wlm"] = np.ascontiguousarray(wlm.reshape(KO, P, V).astype(bf))
    out["wlmT"] = np.ascontiguousarray(wlm.T)
    blm = np.asarray(params["blm"], f32)
    out["blmcol"] = np.ascontiguousarray(blm.reshape(V, 1))
    out["blmrow"] = np.ascontiguousarray(blm.reshape(1, V).astype(bf))
    return out


def kernel(idxs, targets, params):
    idxs = np.asarray(idxs).astype(np.int32)
    targets = np.asarray(targets).astype(np.int32)
    B = idxs.shape[0]
    assert idxs.shape == (B, T) and B == NCORES

    shared = _prep_shared(params)
    has_blm = bool(np.any(np.asarray(params["blm"], np.float32)))
    has_b2 = any(
        bool(np.any(np.asarray(blk["b2"], np.float32)))
        for blk in params["blocks"]
    )
    nc = _get_program(has_blm, has_b2)
    in_maps = []
    for c in range(B):
        m = dict(shared)
        m["idx32"] = np.ascontiguousarray(idxs[c].reshape(T, 1))
        m["tgt32"] = np.ascontiguousarray(targets[c].reshape(T, 1))
        in_maps.append(m)

    trace = bool(int(os.environ.get("BIGRAM_TRACE", "0")))
    if trace:
        try:
            import ntff_shim

            ntff_shim.install()
        except Exception:
            trace = False

    res = run_bass_kernel_spmd(
        nc, in_maps, core_ids=list(range(NCORES)), trace=trace
    )
    if trace and res.exec_time_ns is not None:
        print(f"HW exec time: {res.exec_time_ns} ns")
        if res.instructions_and_trace:
            print("trace:", res.instructions_and_trace[1])

    logits = np.empty((B * T, V), np.float32)
    nll_all = np.empty((B, T), np.float32)
    for c in range(B):
        logits[c * T : (c + 1) * T] = res.results[c]["logits"]
        nll_all[c] = res.results[c]["nll"].reshape(T)
    loss = np.float32(np.mean(nll_all.astype(np.float64)))
    return logits, loss


# revision 11
# speedup vs baseline: 1.0660x; 1.0660x over previous
"""Trainium2 Bass kernel for a 3-layer dense transformer LM (B=8, T=1024,
D=384, H=6, V=32000) returning (logits, loss).

Sharding: pure data-parallel over batch — core c computes batch element c
end-to-end (no collectives). Host assembles logits and averages the
per-token NLLs.

All matmuls run in bf16 with f32 PSUM accumulation. Attention is computed
in transposed-score orientation (scoresT[u,t]) so softmax needs no PE
transposes: the denominator comes from an appended ones-column on V, and
normalization is applied to attT (folded past the linear AV matmul).
"""

import os
import sys

for _p in ("/root/.axon_site/_ro/trn_rl_repo", "/opt/trn_rl_repo"):
    if os.path.isdir(_p) and _p not in sys.path:
        sys.path.append(_p)

import ml_dtypes
import numpy as np

import concourse.bass as bass
import concourse.tile as tile
from concourse import bacc, mybir
from concourse.bass import IndirectOffsetOnAxis
from concourse.bass_utils import run_bass_kernel_spmd
from concourse.masks import make_identity

F32 = mybir.dt.float32
BF16 = mybir.dt.bfloat16
I32 = mybir.dt.int32
AF = mybir.ActivationFunctionType
ALU = mybir.AluOpType

P = 128
D = 384
KO = D // P  # 3
H = 6
HS = 64
T = 1024
NT = T // P  # 8 token tiles
NL = 3
V = 32000
FF = 4 * D  # 1536
FM = FF // P  # 12
G = T // 512  # 2 halves of the token axis
VTW = 512
NVT = (V + VTW - 1) // VTW  # 63 (62 full + 1 x 256)
NCORES = 8
EPS = 1e-5
SCALE = HS**-0.5  # 1/8
NEG_BIG = -1e30


def build_program(has_blm=True, has_b2=True):
    nc = bacc.Bacc(
        "TRN2", target_bir_lowering=False, debug=False, enable_asserts=False
    )

    # ---- DRAM I/O -------------------------------------------------------
    idx32 = nc.dram_tensor("idx32", [T, 1], I32, kind="ExternalInput").ap()
    tgt32 = nc.dram_tensor("tgt32", [T, 1], I32, kind="ExternalInput").ap()
    tok = nc.dram_tensor("tok", [V, D], F32, kind="ExternalInput").ap()
    pos = nc.dram_tensor("pos", [T, D], F32, kind="ExternalInput").ap()

    lw_dram = []
    for l in range(NL):
        lw_dram.append(
            {
                "wq": nc.dram_tensor(f"wq{l}", [KO, P, D], BF16, kind="ExternalInput").ap(),
                "wk": nc.dram_tensor(f"wk{l}", [KO, P, D], BF16, kind="ExternalInput").ap(),
                "wv": nc.dram_tensor(f"wv{l}", [KO, P, D], BF16, kind="ExternalInput").ap(),
                "wo": nc.dram_tensor(f"wo{l}", [H, HS, D], BF16, kind="ExternalInput").ap(),
                "w1": nc.dram_tensor(f"w1_{l}", [KO, P, FF], BF16, kind="ExternalInput").ap(),
                "w2": nc.dram_tensor(f"w2_{l}", [FM, P, D], BF16, kind="ExternalInput").ap(),
                "b1": nc.dram_tensor(f"b1_{l}", [FM, P], F32, kind="ExternalInput").ap(),
                "b2row": nc.dram_tensor(f"b2row{l}", [1, D], BF16, kind="ExternalInput").ap(),
                "g1": nc.dram_tensor(f"g1_{l}", [D], F32, kind="ExternalInput").ap(),
                "be1": nc.dram_tensor(f"be1_{l}", [D], F32, kind="ExternalInput").ap(),
                "g2": nc.dram_tensor(f"g2_{l}", [D], F32, kind="ExternalInput").ap(),
                "be2": nc.dram_tensor(f"be2_{l}", [D], F32, kind="ExternalInput").ap(),
            }
        )
    gf = nc.dram_tensor("gf", [D], F32, kind="ExternalInput").ap()
    bff = nc.dram_tensor("bff", [D], F32, kind="ExternalInput").ap()
    wlm = nc.dram_tensor("wlm", [KO, P, V], BF16, kind="ExternalInput").ap()
    wlmT = nc.dram_tensor("wlmT", [V, D], F32, kind="ExternalInput").ap()
    blmcol = nc.dram_tensor("blmcol", [V, 1], F32, kind="ExternalInput").ap()
    blmrow = nc.dram_tensor("blmrow", [1, V], BF16, kind="ExternalInput").ap()

    logits = nc.dram_tensor("logits", [T, V], F32, kind="ExternalOutput").ap()
    nll = nc.dram_tensor("nll", [T, 1], F32, kind="ExternalOutput").ap()

    with tile.TileContext(nc) as tc:
        _build_body(
            nc, tc, idx32, tgt32, tok, pos, lw_dram, gf, bff, wlm, wlmT,
            blmcol, blmrow, logits, nll, has_blm, has_b2,
        )
    nc.compile()
    return nc


def _build_body(
    nc, tc, idx32, tgt32, tok, pos, lw_dram, gf, bff, wlm, wlmT, blmcol,
    blmrow, logits, nll, has_blm, has_b2,
):
    from contextlib import ExitStack

    ctx = ExitStack()
    with ctx:
        const = ctx.enter_context(tc.tile_pool(name="const", bufs=1))
        lw = ctx.enter_context(tc.tile_pool(name="lw", bufs=1))
        act = ctx.enter_context(tc.tile_pool(name="act", bufs=2))
        big = ctx.enter_context(tc.tile_pool(name="big", bufs=1))
        bigp = ctx.enter_context(tc.tile_pool(name="bigp", bufs=2))
        small = ctx.enter_context(tc.tile_pool(name="small", bufs=4))
        wstream = ctx.enter_context(tc.tile_pool(name="wstream", bufs=3))
        lsb = ctx.enter_context(tc.tile_pool(name="lsb", bufs=4))
        esc = ctx.enter_context(tc.tile_pool(name="esc", bufs=3))
        ps_big = ctx.enter_context(tc.tile_pool(name="ps_big", bufs=5, space="PSUM"))
        ps_att = ctx.enter_context(tc.tile_pool(name="ps_att", bufs=1, space="PSUM"))
        ps_y = ctx.enter_context(tc.tile_pool(name="ps_y", bufs=1, space="PSUM"))
        ps_tr = ctx.enter_context(tc.tile_pool(name="ps_tr", bufs=1, space="PSUM"))

        # ---- constants ---------------------------------------------------
        ident_bf = const.tile([P, P], BF16, tag="ident")
        make_identity(nc, ident_bf[:])
        ones_row = const.tile([1, P], BF16, tag="ones_row")
        nc.vector.memset(ones_row[:], 1.0)
        eps_t = const.tile([P, 1], F32, tag="eps")
        nc.vector.memset(eps_t[:], EPS)
        gfb = const.tile([P, D], F32, tag="gfb")
        nc.sync.dma_start(gfb[:], gf.partition_broadcast(P))
        bfb = const.tile([P, D], F32, tag="bfb")
        nc.sync.dma_start(bfb[:], bff.partition_broadcast(P))

        x_tiles = [const.tile([P, D], F32, tag=f"x{i}", name=f"x{i}") for i in range(NT)]
        s_tiles = [const.tile([P, NVT], F32, tag=f"s{i}", name=f"s{i}") for i in range(NT)]
        tgt_tiles = [const.tile([P, 1], F32, tag=f"tgt{i}", name=f"tgt{i}") for i in range(NT)]

        # ---- helpers -----------------------------------------------------
        def layernorm(src_ap, g_b, be_b, out_ap):
            """out = (src - mean)/sqrt(var+eps) * g + be, written to out_ap."""
            stats = small.tile([P, 6], F32, tag="bnstats")
            nc.vector.bn_stats(stats[:], src_ap)
            mv = small.tile([P, 2], F32, tag="mv")
            nc.vector.bn_aggr(mv[:], stats[:])
            rstd = small.tile([P, 1], F32, tag="rstd")
            nc.scalar.activation(rstd[:], mv[:, 1:2], AF.Sqrt, bias=eps_t[:], scale=1.0)
            nc.vector.reciprocal(rstd[:], rstd[:])
            nmr = small.tile([P, 1], F32, tag="nmr")
            nc.vector.tensor_tensor(nmr[:], mv[:, 0:1], rstd[:], ALU.mult)
            nc.vector.tensor_scalar_mul(nmr[:], nmr[:], -1.0)
            htmp = act.tile([P, D], F32, tag="htmp")
            nc.scalar.activation(htmp[:], src_ap, AF.Identity, bias=nmr[:], scale=rstd[:])
            nc.vector.tensor_tensor(htmp[:], htmp[:], g_b[:], ALU.mult)
            nc.vector.tensor_tensor(out_ap, htmp[:], be_b[:], ALU.add)

        def transpose_to(dst_ap, src_ap):
            """dst[128,128] (bf16 sbuf) = src[128,128] (bf16 sbuf) transposed.

            Regular matmul against the identity (out = src.T @ I): the
            dedicated is_transpose path requires a bf16 PSUM tile, which
            crashes the device (NRT_EXEC_UNIT_UNRECOVERABLE)."""
            pt = ps_tr.tile([P, P], F32, tag="ptr", space="PSUM")
            nc.tensor.matmul(pt[:], lhsT=src_ap, rhs=ident_bf[:], start=True, stop=True)
            nc.vector.tensor_copy(dst_ap, pt[:])

        # ---- embedding ---------------------------------------------------
        es = nc.named_scope("embed"); es.__enter__()
        for i in range(NT):
            sl = slice(i * P, (i + 1) * P)
            idx_t = small.tile([P, 1], I32, tag="idx")
            nc.sync.dma_start(idx_t[:], idx32[sl, :])
            xg = act.tile([P, D], F32, tag="xg")
            nc.gpsimd.indirect_dma_start(
                out=xg[:],
                out_offset=None,
                in_=tok[:],
                in_offset=IndirectOffsetOnAxis(ap=idx_t[:, :1], axis=0),
            )
            pos_t = act.tile([P, D], F32, tag="pos_t")
            nc.sync.dma_start(pos_t[:], pos[sl, :])
            nc.vector.tensor_tensor(x_tiles[i][:], xg[:], pos_t[:], ALU.add)

        es.__exit__(None, None, None)
        # ---- transformer layers -----------------------------------------
        for l in range(NL):
            ls = nc.named_scope(f"layer{l}"); ls.__enter__()
            d = lw_dram[l]
            wq_sb = lw.tile([P, KO, D], BF16, tag="wq")
            nc.sync.dma_start(wq_sb[:], d["wq"].rearrange("ko p n -> p ko n"))
            wk_sb = lw.tile([P, KO, D], BF16, tag="wk")
            nc.sync.dma_start(wk_sb[:], d["wk"].rearrange("ko p n -> p ko n"))
            wv_sb = lw.tile([P, KO, D], BF16, tag="wv")
            nc.sync.dma_start(wv_sb[:], d["wv"].rearrange("ko p n -> p ko n"))
            wo_sb = lw.tile([HS, H, D], BF16, tag="wo")
            nc.sync.dma_start(wo_sb[:], d["wo"].rearrange("h s n -> s h n"))
            w1_sb = lw.tile([P, KO, FF], BF16, tag="w1")
            nc.sync.dma_start(w1_sb[:], d["w1"].rearrange("ko p n -> p ko n"))
            w2_sb = lw.tile([P, FM, D], BF16, tag="w2")
            nc.sync.dma_start(w2_sb[:], d["w2"].rearrange("m p n -> p m n"))
            b1_sb = lw.tile([P, FM], F32, tag="b1")
            nc.sync.dma_start(b1_sb[:], d["b1"].rearrange("m p -> p m"))
            b2row_sb = lw.tile([1, D], BF16, tag="b2row")
            nc.sync.dma_start(b2row_sb[:], d["b2row"][:])
            g1b = lw.tile([P, D], F32, tag="g1b")
            nc.sync.dma_start(g1b[:], d["g1"].partition_broadcast(P))
            be1b = lw.tile([P, D], F32, tag="be1b")
            nc.sync.dma_start(be1b[:], d["be1"].partition_broadcast(P))
            g2b = lw.tile([P, D], F32, tag="g2b")
            nc.sync.dma_start(g2b[:], d["g2"].partition_broadcast(P))
            be2b = lw.tile([P, D], F32, tag="be2b")
            nc.sync.dma_start(be2b[:], d["be2"].partition_broadcast(P))

            # LN1 + transpose h into hT, interleaved with qkv per half so
            # the PE never idles a full HAM window during the LN chain
            hT = big.tile([P, KO, T], BF16, tag="hT")
            qT = big.tile([P, KO, T], BF16, tag="qT")
            kT = big.tile([P, KO, T], BF16, tag="kT")
            vaug = big.tile([P, NT, H, HS + 1], BF16, tag="vaug")
            nc.vector.memset(vaug[:, :, :, HS : HS + 1], 1.0)
            for g in range(G):
                for i in range(4 * g, 4 * g + 4):
                    h_bf = act.tile([P, D], BF16, tag="h_bf")
                    layernorm(x_tiles[i][:], g1b, be1b, h_bf[:])
                    for ko in range(KO):
                        transpose_to(
                            hT[:, ko, i * P : (i + 1) * P],
                            h_bf[:, ko * P : (ko + 1) * P],
                        )
                for w_sb, oT in ((wq_sb, qT), (wk_sb, kT)):
                    for m in range(KO):  # head pair m -> heads 2m, 2m+1
                        psq = ps_big.tile([P, 512], F32, tag="ps512", space="PSUM")
                        for ko in range(KO):
                            nc.tensor.matmul(
                                psq[:],
                                lhsT=w_sb[:, ko, m * P : (m + 1) * P],
                                rhs=hT[:, ko, g * 512 : (g + 1) * 512],
                                start=(ko == 0),
                                stop=(ko == KO - 1),
                            )
                        gs = slice(g * 512, (g + 1) * 512)
                        nc.scalar.copy(oT[:, m, gs], psq[:])
                for i in range(4 * g, 4 * g + 4):
                    psv_full = ps_big.tile([P, 512], F32, tag="ps512", space="PSUM")
                    psv = psv_full[:, :D]
                    for ko in range(KO):
                        nc.tensor.matmul(
                            psv[:],
                            lhsT=hT[:, ko, i * P : (i + 1) * P],
                            rhs=wv_sb[:, ko, :],
                            start=(ko == 0),
                            stop=(ko == KO - 1),
                        )
                    nc.vector.tensor_copy(
                        vaug[:, i, :, 0:HS],
                        psv[:].rearrange("p (h s) -> p h s", h=H),
                    )

            # attention, head by head
            attT = big.tile([HS, H, T], BF16, tag="attT")
            for h in range(H):
                probsT = bigp.tile(
                    [P, NT, T], BF16, tag="probsT", name=f"probsT{h}"
                )
                for g in range(G):
                    jmax = 4 * g + 3
                    gs = slice(g * 512, (g + 1) * 512)
                    for j in range(jmax + 1):
                        tlo = max(g * 512, j * P)
                        n = (g + 1) * 512 - tlo
                        pss = ps_big.tile([P, 512], F32, tag="ps512", space="PSUM")
                        po = (h % 2) * HS
                        m_h = h // 2
                        nc.tensor.matmul(
                            pss[:, :n],
                            lhsT=kT[po : po + HS, m_h, j * P : (j + 1) * P],
                            rhs=qT[po : po + HS, m_h, tlo : (g + 1) * 512],
                            start=True,
                            stop=True,
                        )
                        if j * P >= g * 512:  # diagonal block present
                            if j * P > g * 512:
                                nc.vector.memset(probsT[:, j, g * 512 : j * P], 0.0)
                            sdiag = small.tile([P, P], F32, tag="sdiag")
                            nc.scalar.copy(sdiag[:], pss[:, 0:P])
                            nc.gpsimd.affine_select(
                                out=sdiag[:],
                                in_=sdiag[:],
                                pattern=[[1, P]],
                                compare_op=ALU.is_ge,
                                fill=NEG_BIG,
                                base=0,
                                channel_multiplier=-1,
                            )
                            nc.scalar.activation(
                                probsT[:, j, tlo : tlo + P], sdiag[:], AF.Exp,
                                scale=SCALE,
                            )
                            if n > P:
                                nc.scalar.activation(
                                    probsT[:, j, tlo + P : (g + 1) * 512],
                                    pss[:, P:n], AF.Exp, scale=SCALE,
                                )
                        else:
                            nc.scalar.activation(
                                probsT[:, j, gs], pss[:, :n], AF.Exp, scale=SCALE
                            )
                    # AV: attT_aug[s,t] (+ rowsum in row HS) over u blocks
                    psa = ps_att.tile([HS + 1, 512], F32, tag="psa", space="PSUM")
                    for j in range(jmax + 1):
                        nc.tensor.matmul(
                            psa[:],
                            lhsT=vaug[:, j, h, :],
                            rhs=probsT[:, j, gs],
                            start=(j == 0),
                            stop=(j == jmax),
                        )
                    rs_row = small.tile([1, 512], F32, tag="rs_row")
                    nc.scalar.copy(rs_row[:], psa[HS : HS + 1, :])
                    rsB = small.tile([HS, 512], F32, tag="rsB")
                    nc.gpsimd.partition_broadcast(rsB[:], rs_row[:], channels=HS)
                    nc.vector.reciprocal(rsB[:], rsB[:])
                    nc.vector.tensor_tensor(
                        attT[:, h, gs], psa[0:HS, :], rsB[:], ALU.mult
                    )

            # output projection + residual
            for i in range(NT):
                psy = ps_y.tile([P, D], F32, tag="psy", space="PSUM")
                for h in range(H):
                    nc.tensor.matmul(
                        psy[:],
                        lhsT=attT[:, h, i * P : (i + 1) * P],
                        rhs=wo_sb[:, h, :],
                        start=(h == 0),
                        stop=(h == H - 1),
                    )
                nc.vector.tensor_tensor(x_tiles[i][:], x_tiles[i][:], psy[:], ALU.add)

            # FFW (LN2+transposes interleaved with ff1 per half)
            h2T = big.tile([P, KO, T], BF16, tag="hT")  # reuse hT slot
            ff1T = big.tile([P, FM, T], BF16, tag="ff1T")
            for g in range(G):
                for i in range(4 * g, 4 * g + 4):
                    h2_bf = act.tile([P, D], BF16, tag="h_bf")
                    layernorm(x_tiles[i][:], g2b, be2b, h2_bf[:])
                    for ko in range(KO):
                        transpose_to(
                            h2T[:, ko, i * P : (i + 1) * P],
                            h2_bf[:, ko * P : (ko + 1) * P],
                        )
                for m in range(FM):
                    psf = ps_big.tile([P, 512], F32, tag="ps512", space="PSUM")
                    for ko in range(KO):
                        nc.tensor.matmul(
                            psf[:],
                            lhsT=w1_sb[:, ko, m * P : (m + 1) * P],
                            rhs=h2T[:, ko, g * 512 : (g + 1) * 512],
                            start=(ko == 0),
                            stop=(ko == KO - 1),
                        )
                    nc.scalar.activation(
                        ff1T[:, m, g * 512 : (g + 1) * 512], psf[:], AF.Relu,
                        bias=b1_sb[:, m : m + 1], scale=1.0,
                    )
            for i in range(NT):
                psy2 = ps_y.tile([P, D], F32, tag="psy", space="PSUM")
                for m in range(FM):
                    nc.tensor.matmul(
                        psy2[:],
                        lhsT=ff1T[:, m, i * P : (i + 1) * P],
                        rhs=w2_sb[:, m, :],
                        start=(m == 0),
                        stop=(not has_b2 and m == FM - 1),
                    )
                if has_b2:
                    nc.tensor.matmul(
                        psy2[:], lhsT=ones_row[:], rhs=b2row_sb[:], start=False,
                        stop=True,
                    )
                nc.vector.tensor_tensor(x_tiles[i][:], x_tiles[i][:], psy2[:], ALU.add)

            ls.__exit__(None, None, None)
        # ---- final LN, target-logit dot ---------------------------------
        fs = nc.named_scope("finalln"); fs.__enter__()
        hfT = big.tile([P, KO, T], BF16, tag="hT")
        for i in range(NT):
            sl = slice(i * P, (i + 1) * P)
            hf = act.tile([P, D], F32, tag="hf")
            layernorm(x_tiles[i][:], gfb, bfb, hf[:])
            hf_bf = act.tile([P, D], BF16, tag="h_bf")
            nc.vector.tensor_copy(hf_bf[:], hf[:])
            for ko in range(KO):
                transpose_to(
                    hfT[:, ko, i * P : (i + 1) * P], hf_bf[:, ko * P : (ko + 1) * P]
                )
            tgi = small.tile([P, 1], I32, tag="idx")
            nc.sync.dma_start(tgi[:], tgt32[sl, :])
            wg = act.tile([P, D], F32, tag="wg")
            nc.gpsimd.indirect_dma_start(
                out=wg[:],
                out_offset=None,
                in_=wlmT[:],
                in_offset=IndirectOffsetOnAxis(ap=tgi[:, :1], axis=0),
            )
            bg = small.tile([P, 1], F32, tag="bg")
            nc.gpsimd.indirect_dma_start(
                out=bg[:],
                out_offset=None,
                in_=blmcol[:],
                in_offset=IndirectOffsetOnAxis(ap=tgi[:, :1], axis=0),
            )
            ttr_out = act.tile([P, D], F32, tag="ttr")
            tdot = small.tile([P, 1], F32, tag="tdot")
            nc.vector.tensor_tensor(ttr_out[:], hf[:], wg[:], ALU.mult)
            nc.vector.tensor_reduce(
                tdot[:], ttr_out[:], axis=mybir.AxisListType.X, op=ALU.add
            )
            nc.vector.tensor_tensor(tgt_tiles[i][:], tdot[:], bg[:], ALU.add)

        fs.__exit__(None, None, None)
        hs = nc.named_scope("lmhead"); hs.__enter__()
        for vt in range(NVT):
            c0 = vt * VTW
            w = min(VTW, V - c0)
            wlm_t = wstream.tile([P, KO, VTW], BF16, tag="wlm")
            nc.sync.dma_start(
                wlm_t[:, :, :w],
                wlm[:, :, c0 : c0 + w].rearrange("ko p n -> p ko n"),
            )
            if has_blm:
                blm_t = wstream.tile([1, VTW], BF16, tag="blmr")
                nc.sync.dma_start(blm_t[:, :w], blmrow[:, c0 : c0 + w])
            for i in range(NT):
                psl = ps_big.tile([P, VTW], F32, tag="ps512", space="PSUM")
                for ko in range(KO):
                    nc.tensor.matmul(
                        psl[:, :w],
                        lhsT=hfT[:, ko, i * P : (i + 1) * P],
                        rhs=wlm_t[:, ko, :w],
                        start=(ko == 0),
                        stop=(not has_blm and ko == KO - 1),
                    )
                if has_blm:
                    nc.tensor.matmul(
                        psl[:, :w],
                        lhsT=ones_row[:],
                        rhs=blm_t[:, :w],
                        start=False,
                        stop=True,
                    )
                lt = lsb.tile([P, VTW], F32, tag="lt")
                nc.vector.tensor_copy(lt[:, :w], psl[:, :w])
                nc.sync.dma_start(logits[i * P : (i + 1) * P, c0 : c0 + w], lt[:, :w])
                et = esc.tile([P, VTW], BF16, tag="et")
                nc.scalar.activation(
                    et[:, :w], psl[:, :w], AF.Exp,
                    accum_out=s_tiles[i][:, vt : vt + 1],
                )

        hs.__exit__(None, None, None)
        ts_ = nc.named_scope("loss"); ts_.__enter__()
        for i in range(NT):
            ssum = small.tile([P, 1], F32, tag="ssum")
            nc.vector.tensor_reduce(
                ssum[:], s_tiles[i][:], axis=mybir.AxisListType.X, op=ALU.add
            )
            lse = small.tile([P, 1], F32, tag="lse")
            nc.scalar.activation(lse[:], ssum[:], AF.Ln)
            nll_t = small.tile([P, 1], F32, tag="nll_t")
            nc.vector.tensor_tensor(nll_t[:], lse[:], tgt_tiles[i][:], ALU.subtract)
            nc.sync.dma_start(nll[i * P : (i + 1) * P, :], nll_t[:])
        ts_.__exit__(None, None, None)


# ---------------------------------------------------------------------------
# Host side
# ---------------------------------------------------------------------------

_PROGRAMS = {}


def _get_program(has_blm, has_b2):
    key = (has_blm, has_b2)
    if key not in _PROGRAMS:
        _PROGRAMS[key] = build_program(has_blm, has_b2)
    return _PROGRAMS[key]


def _prep_shared(params):
    bf = ml_dtypes.bfloat16
    f32 = np.float32
    out = {}
    out["tok"] = np.ascontiguousarray(np.asarray(params["tok"], f32))
    out["pos"] = np.ascontiguousarray(np.asarray(params["pos"], f32)[:T])
    for l, blk in enumerate(params["blocks"]):
        for nm in ("wq", "wk", "wv"):
            w = np.asarray(blk[nm], f32)  # [H, D, HS]
            w = np.transpose(w, (1, 0, 2)).reshape(D, H * HS)
            out[f"{nm}{l}"] = np.ascontiguousarray(w.reshape(KO, P, D).astype(bf))
        wo = np.asarray(blk["wo"], f32).reshape(H, HS, D)
        out[f"wo{l}"] = np.ascontiguousarray(wo.astype(bf))
        out[f"w1_{l}"] = np.ascontiguousarray(
            np.asarray(blk["w1"], f32).reshape(KO, P, FF).astype(bf)
        )
        out[f"w2_{l}"] = np.ascontiguousarray(
            np.asarray(blk["w2"], f32).reshape(FM, P, D).astype(bf)
        )
        out[f"b1_{l}"] = np.ascontiguousarray(
            np.asarray(blk["b1"], f32).reshape(FM, P)
        )
        out[f"b2row{l}"] = np.ascontiguousarray(
            np.asarray(blk["b2"], f32).reshape(1, D).astype(bf)
        )
        out[f"g1_{l}"] = np.ascontiguousarray(np.asarray(blk["g1"], f32))
        out[f"be1_{l}"] = np.ascontiguousarray(np.asarray(blk["be1"], f32))
        out[f"g2_{l}"] = np.ascontiguousarray(np.asarray(blk["g2"], f32))
        out[f"be2_{l}"] = np.ascontiguousarray(np.asarray(blk["be2"], f32))
    out["gf"] = np.ascontiguousarray(np.asarray(params["gf"], f32))
    out["bff"] = np.ascontiguousarray(np.asarray(params["bf"], f32))
    wlm = np.asarray(params["wlm"], f32)  # [D, V]
    out["wlm"] = np.ascontiguousarray(wlm.reshape(KO, P, V).astype(bf))
    out["wlmT"] = np.ascontiguousarray(wlm.T)
    blm = np.asarray(params["blm"], f32)
    out["blmcol"] = np.ascontiguousarray(blm.reshape(V, 1))
    out["blmrow"] = np.ascontiguousarray(blm.reshape(1, V).astype(bf))
    return out


def kernel(idxs, targets, params):
    idxs = np.asarray(idxs).astype(np.int32)
    targets = np.asarray(targets).astype(np.int32)
    B = idxs.shape[0]
    assert idxs.shape == (B, T) and B == NCORES

    shared = _prep_shared(params)
    has_blm = bool(np.any(np.asarray(params["blm"], np.float32)))
    has_b2 = any(
        bool(np.any(np.asarray(blk["b2"], np.float32)))
        for blk in params["blocks"]
    )
    nc = _get_program(has_blm, has_b2)
    in_maps = []
    for c in range(B):
        m = dict(shared)
        m["idx32"] = np.ascontiguousarray(idxs[c].reshape(T, 1))
        m["tgt32"] = np.ascontiguousarray(targets[c].reshape(T, 1))
        in_maps.append(m)

    trace = bool(int(os.environ.get("BIGRAM_TRACE", "0")))
    if trace:
        try:
            import ntff_shim

            ntff_shim.install()
        except Exception:
            trace = False

    res = run_bass_kernel_spmd(
        nc, in_maps, core_ids=list(range(NCORES)), trace=trace
    )
    if trace and res.exec_time_ns is not None:
        print(f"HW exec time: {res.exec_time_ns} ns")
        if res.instructions_and_trace:
            print("trace:", res.instructions_and_trace[1])

    logits = np.empty((B * T, V), np.float32)
    nll_all = np.empty((B, T), np.float32)
    for c in range(B):
        logits[c * T : (c + 1) * T] = res.results[c]["logits"]
        nll_all[c] = res.results[c]["nll"].reshape(T)
    loss = np.float32(np.mean(nll_all.astype(np.float64)))
    return logits, loss
